# revision 87
# baseline (speedup 1.0000x reference)
"""CPGNN (compatibility-guided GNN) kernel for 8 Trainium2 NeuronCores.

Reference computation (N=10000, F=512, HID=256, C=16, 4 post iterations):
    h      = relu(normed_adj @ (features @ W1) + b1)
    logits = normed_adj @ (h @ W2) + b2
    E_hat  = softmax(logits) - 1/C
    B_hat  = E_hat;  4x: B_hat = E_hat + raw_adj @ (B_hat @ H)
    out    = B_hat + 1/C

Strategy (v2):
  * Row-shard both adjacencies over the 8 cores (1280 rows/core, padded).
    Shards are uploaded K-major ([10240, 1280]) in fp8-e4m3 with power-of-2
    pre-scaling, and every big N x N sweep runs as fp8 DoubleRow matmuls
    (256-deep contraction per instruction, 2x PE throughput).
  * Stationary operands (XW1, hW2, B_hat) are quantized as fp8 residual
    PAIRS (q1 = fp8(x), q2 = fp8(x - q1)) placed side by side in the
    stationary, which costs nothing extra in the moving-bound sweeps and is
    more accurate than bf16.
  * raw_adj @ (B @ H) is computed as (raw_adj @ B) @ H: the narrow sweep
    produces Z = R@B c-major, then one tiny f32r matmul per m-tile applies
    [H;H] (summing the q1/q2 parts) with all de/re-quantization scales
    folded in, directly in K-major layout for the next iteration.
  * adjn is fully SBUF-resident (loaded once, reused by phases 2 and 4);
    adjr is cached during iteration 0's stream (R_RES double-tiles) and the
    rest re-streamed per iteration, when the DMA pipe is otherwise idle.
  * Phase 1 (X @ W1, bf16) is sharded: each core computes its 1280-row
    slice from a per-core xT shard and the fp8 pair is all-gathered.
  * The small [N,C]-pair tensors are all-gathered (40KB fp8/rank) between
    iterations, with PE warm-keeper chains bridging the collectives and
    p-major gather layouts for fast reloads.
"""

import os

import numpy as np
import ml_dtypes

RANKS = 8
P = 128
NREAL = 10000
NK = 10240            # padded global row count (80 k-tiles, 40 double-tiles)
ML = 1280             # local rows per core (10 m-tiles)
KT = NK // P          # 80
DT = KT // 2          # 40 double (k=256) tiles
MT = ML // P          # 10
F = 512
FT = F // P           # 4
HID = 256
C = 16
C2 = 2 * C
NPOST = 4
CH = [(0, 512), (512, 1024), (1024, 1280)]

A_RES = 40            # adjn fully resident in SBUF (loaded once, ph2 + ph4)
R_RES = 27            # adjr double-tiles cached during iter 0's stream

# power-of-2 quantization scales (validated against reference numerics)
S_N = 4096.0          # normed_adj pre-scale (host)
S_R = 8192.0          # raw_adj pre-scale (host)
S_X = 16.0            # XW1 pair scale
S_W = 64.0            # hW2 pair scale
S_B = [65536.0, 2048.0, 128.0, 8.0]   # B_hat pair scale at iteration input

# build-time switch: single-core TimelineSim build (collectives -> local DMA)
TLSIM = os.environ.get("CPGNN_TLSIM", "0") == "1"

_CACHE = {}


def _mix_order(n_cache, n_total):
    """Interleave cached (0..n_cache-1) and streamed (n_cache..) indices so
    DMA of streamed tiles overlaps PE work on cached tiles evenly."""
    cached = list(range(n_cache))
    streamed = list(range(n_cache, n_total))
    order = []
    ic = si = 0
    for i in range(n_total):
        want_stream = streamed and (si + 1) / len(streamed) <= (i + 1) / n_total
        if si < len(streamed) and (ic >= len(cached) or want_stream):
            order.append(streamed[si]); si += 1
        else:
            order.append(cached[ic]); ic += 1
    assert sorted(order) == list(range(n_total))
    return order


def _build_and_compile():
    import concourse.mybir as mybir
    import concourse.tile as tile
    import concourse.bass as bass_mod
    from concourse import bacc

    dt = mybir.dt
    f32 = dt.float32
    bf16 = dt.bfloat16
    f8 = dt.float8e4
    AF = mybir.ActivationFunctionType
    DR = mybir.MatmulPerfMode.DoubleRow

    nc = bacc.Bacc("TRN2", target_bir_lowering=False, debug=False,
                   num_devices=1 if TLSIM else RANKS)

    def all_gather(loc, full, name):
        """AllGather p-major pair shards [P, MT*C2] -> [P*RANKS, MT*C2];
        local-DMA stub under TLSIM."""
        if TLSIM:
            nc.scalar.dma_start(full[0:P, :], loc[:])
        else:
            nc.gpsimd.collective_compute(
                "AllGather", mybir.AluOpType.bypass,
                replica_groups=[list(range(RANKS))],
                ins=[loc[:].opt()], outs=[full[:].opt()])

    adjn8 = nc.dram_tensor("adjn8", [NK, ML], f8, kind="ExternalInput").ap()
    adjr8 = nc.dram_tensor("adjr8", [NK, ML], f8, kind="ExternalInput").ap()
    xTs = nc.dram_tensor("xTs", [F, ML], bf16, kind="ExternalInput").ap()
    w1 = nc.dram_tensor("w1", [F, HID], bf16, kind="ExternalInput").ap()
    w2 = nc.dram_tensor("w2", [HID, C], bf16, kind="ExternalInput").ap()
    hs = nc.dram_tensor("hs", [NPOST * 2 * C, C], f32,
                        kind="ExternalInput").ap()
    b1 = nc.dram_tensor("b1", [HID, 1], f32, kind="ExternalInput").ap()
    b2k = nc.dram_tensor("b2k", [P, MT * C], f32, kind="ExternalInput").ap()
    ident = nc.dram_tensor("ident", [2 * C, 2 * C], f32,
                           kind="ExternalInput").ap()
    outKM = nc.dram_tensor("outKM", [ML, C], f32, kind="ExternalOutput").ap()

    adjn_r = adjn8.rearrange("(t i p) m -> p t i m", i=2, p=P)
    adjr_r = adjr8.rearrange("(t i p) m -> p t i m", i=2, p=P)

    with tile.TileContext(nc) as tc:
        with tc.tile_pool(name="const", bufs=1) as const_pool, \
             tc.tile_pool(name="persist", bufs=1) as persist, \
             tc.tile_pool(name="adjn_cm", bufs=1) as adjn_cm, \
             tc.tile_pool(name="strm", bufs=5) as strm, \
             tc.tile_pool(name="dram", bufs=1, space="DRAM") as dram_pool:

            # ---- constants ----
            # w1 on the SP queue (needed immediately); the small phase-3/4/5
            # constants go on the Activation HWDGE queue to keep SP free for
            # the big streams.
            w1_sb = const_pool.tile([P, FT, HID], bf16)
            nc.scalar.dma_start(w1_sb[:],
                                w1.rearrange("(kf p) h -> p kf h", p=P))

            # ---- persistent intermediates ----
            h1t_sb = persist.tile([P, 2, ML], bf16)        # h.T  [HID, ML]
            hw2f_sb = persist.tile([P, DT, 2, C2], f8)     # gathered hW2 pair
            btq_sb = persist.tile([P, DT, 2, C2], f8)      # gathered B pair
            e_kmaj = persist.tile([P, MT, C], f32)         # E K-major local
            zb_sb = persist.tile([2 * C, ML], f32)         # sweep out copy

            # adjn residency (phase 2 + phase 4): one tile per double-tile so
            # a consumer only waits on its own DMA, not all 40
            adjn_res = [adjn_cm.tile([P, 2, ML], f8, name=f"adjn_res{d}")
                        for d in range(A_RES)]

            # p-major pair gather helpers: local [P, MT, C2] -> DRAM
            # [P, MT*C2] -> AG stacks ranks on dim0 -> [P*RANKS, MT*C2];
            # reload as [P, DT, 2, C2] with 8x320B runs per partition.
            def gather_pair(pair_sb, dst_sb, name):
                loc = dram_pool.tile([P, MT * C2], f8, name=f"{name}_loc")
                nc.scalar.dma_start(
                    loc.rearrange("p (m c) -> p m c", c=C2), pair_sb[:])
                full = dram_pool.tile([P * RANKS, MT * C2], f8,
                                      name=f"{name}_full",
                                      addr_space="Shared")
                all_gather(loc, full, name)
                # bulk reload on the (idle) gpsimd SWDGE queue so it doesn't
                # block the Activation instruction stream
                nc.gpsimd.dma_start(
                    dst_sb.rearrange("p (r j) i c -> p r j i c", r=RANKS),
                    full.rearrange("(r p) (j i c) -> p r j i c",
                                   p=P, i=2, c=C2))

            # keep the PE array warm (HAM) during a collective: a chain of
            # tiny fp8 matmuls, each gated on an ACT copy of the previous
            # (PE->ACT->PE ~0.8us/link), spreading over the gather window.
            def warm_pe(ps_pool, sb_pool, dep, mov, n=12):
                scr = ps_pool.tile([2 * C, 512], f32, name="warm_ps", bufs=1)
                tok = sb_pool.tile([2 * C, 2 * C], f8, name="warm_tok",
                                   bufs=1)
                st = dep
                for j in range(n):
                    nc.tensor.matmul(scr[:], st, mov, start=True, stop=True)
                    nc.vector.tensor_copy(tok[:], scr[:, 0:2 * C])
                    st = tok[:]

            # ============ phase 1 (sharded): local XW1 slice = Xs @ W1,
            # fp8 residual pair, all-gathered K-major across the 8 cores.
            H2 = 2 * HID
            JL = DT // RANKS                   # 5 local double-tiles
            xw_cm = tc.tile_pool(name="xw_cm", bufs=1)
            xw_pool = xw_cm.__enter__()
            xw1q_sb = [xw_pool.tile([P, JL, 2, H2], f8, name=f"xw1q{r}")
                       for r in range(RANKS)]

            with tc.tile_pool(name="ph1p", bufs=1) as ph1p, \
                 tc.tile_pool(name="ph1w", bufs=2) as ph1w, \
                 tc.tile_pool(name="ps1", bufs=4, space="PSUM") as ps1:
                xts = ph1p.tile([P, FT, ML], bf16)
                nc.sync.dma_start(xts[:],
                                  xTs.rearrange("(kf p) n -> p kf n", p=P))
                # resident adjn prefetch, head only (tail issued on the
                # gpsimd queue after the xw1q loads so they don't hog the
                # DMA pipe during the xw gather)
                for dtile in range(15):
                    nc.sync.dma_start(adjn_res[dtile][:], adjn_r[:, dtile])
                xwloc = ph1p.tile([P, JL, 2, H2], f8)
                xw_loc_dram = dram_pool.tile([P, JL * 2 * H2], f8,
                                             name="xw_loc")
                for j in range(JL):
                    for i in range(2):
                        ktl = 2 * j + i
                        psum1 = ps1.tile([P, HID], f32, name="psum1")
                        for kf in range(FT):
                            nc.tensor.matmul(
                                psum1[:],
                                xts[:, kf, ktl * P:(ktl + 1) * P],
                                w1_sb[:, kf, :],
                                start=(kf == 0), stop=(kf == FT - 1))
                        nc.scalar.activation(xwloc[:, j, i, 0:HID], psum1[:],
                                             AF.Copy, scale=S_X)
                        nc.vector.scalar_tensor_tensor(
                            xwloc[:, j, i, HID:H2], psum1[:], S_X,
                            xwloc[:, j, i, 0:HID],
                            mybir.AluOpType.mult, mybir.AluOpType.subtract)

                nc.scalar.dma_start(
                    xw_loc_dram.rearrange("p (j i q) -> p j i q", i=2, q=H2),
                    xwloc[:])
                xw_full = dram_pool.tile([P * RANKS, JL * 2 * H2], f8,
                                         name="xw_full", addr_space="Shared")
                all_gather(xw_loc_dram, xw_full, "xw")
                # per-rank reload (separate tiles) so phase 2 starts as
                # soon as rank-0's block lands; gpsimd queue keeps ACT clear
                xwf_v = xw_full.rearrange("(r p) (j i q) -> p r j i q",
                                          p=P, i=2, q=H2)
                for r in range(RANKS):
                    nc.gpsimd.dma_start(xw1q_sb[r][:], xwf_v[:, r])
                # gate the adjn tail loads behind the first xw1q blocks via
                # DVE FIFO: g1 waits the rank-2 load; the tiny tail reads
                # queue behind g1, and each tail DMA WAR-waits its read. This
                # keeps the DMA pipe clear for the xw gather chain.
                scrap = ph1w.tile([2 * C, 2 * C], f8, name="scrap", bufs=1)
                for dtile in range(15, A_RES):
                    nc.gpsimd.memset(adjn_res[dtile][0:2, 0, 0:2], 0.0)
                nc.vector.tensor_copy(scrap[:],
                                      xw1q_sb[2][0:2 * C, 0, 0, 0:2 * C])
                for dtile in range(15, A_RES):
                    nc.vector.tensor_copy(scrap[0:2, 0:2],
                                          adjn_res[dtile][0:2, 0, 0:2])
                for dtile in range(15, A_RES):
                    nc.sync.dma_start(adjn_res[dtile][:], adjn_r[:, dtile])
                warm_pe(ps1, ph1w, xwloc[0:2 * C, 0, 0, 0:2 * C],
                        adjn_res[0][0:2 * C, 0, 0:512], n=10)

            # late-phase constants (issued here so their descriptor-gen
            # doesn't clog the Activation stream during phase 1)
            w2_sb = const_pool.tile([P, 2, C], bf16)
            nc.scalar.dma_start(w2_sb[:],
                                w2.rearrange("(mh p) c -> p mh c", p=P))
            hs_sb = const_pool.tile([2 * C, NPOST, C], f32)
            nc.scalar.dma_start(hs_sb[:],
                                hs.rearrange("(n q) c -> q n c", q=2 * C))
            b1_sb = const_pool.tile([P, 2, 1], f32)
            nc.scalar.dma_start(b1_sb[:],
                                b1.rearrange("(t p) o -> p t o", p=P))
            b2k_sb = const_pool.tile([P, MT, C], f32)
            nc.scalar.dma_start(b2k_sb[:],
                                b2k.rearrange("p (m c) -> p m c", c=C))
            ident_sb = const_pool.tile([2 * C, 2 * C], f32)
            nc.scalar.dma_start(ident_sb[:], ident[:])

            # ============ phase 2: h.T = relu((XW1 pair).T @DR adjn + b1)
            with tc.tile_pool(name="ps2", bufs=1, space="PSUM") as ps2:
                psum_h0 = ps2.tile([P, ML], f32, name="psum_h0")
                psum_h1 = ps2.tile([P, ML], f32, name="psum_h1")
                psum_h = [psum_h0, psum_h1]
                for dtile in range(DT):
                    src = adjn_res[dtile]
                    for mh in range(2):
                        for qi in range(2):
                            off = qi * HID + mh * P
                            for (n0, n1) in CH:
                                nc.tensor.matmul(
                                    psum_h[mh][:, n0:n1],
                                    xw1q_sb[dtile // JL][:, dtile % JL,
                                                         :, off:off + P],
                                    src[:, :, n0:n1],
                                    start=(dtile == 0 and qi == 0),
                                    stop=(dtile == DT - 1 and qi == 1),
                                    perf_mode=DR)
                for mh in range(2):
                    nc.scalar.activation(h1t_sb[:, mh, :], psum_h[mh][:],
                                         AF.Relu, bias=b1_sb[:, mh, :],
                                         scale=1.0 / (S_X * S_N))
            xw_cm.__exit__(None, None, None)

            # adjr cache lives in the space freed by the xw1 pair
            adjr_cm = tc.tile_pool(name="adjr_cm", bufs=1)
            adjr_pool = adjr_cm.__enter__()
            adjr_res = [adjr_pool.tile([P, 2, ML], f8, name=f"adjr_res{d}")
                        for d in range(R_RES)]

            # ============ phase 3: hW2 = h @ W2 (K-major), pair, all-gather
            with tc.tile_pool(name="ph3", bufs=1) as ph3, \
                 tc.tile_pool(name="ps3", bufs=1, space="PSUM") as ps3:
                psum_hw = ps3.tile([P, MT, C], f32)
                for m in range(MT):
                    for mh in range(2):
                        nc.tensor.matmul(
                            psum_hw[:, m, :],
                            h1t_sb[:, mh, m * P:(m + 1) * P],
                            w2_sb[:, mh, :],
                            start=(mh == 0), stop=(mh == 1))
                hq_sb = ph3.tile([P, MT, C2], f8)
                nc.scalar.activation(hq_sb[:, :, 0:C], psum_hw[:], AF.Copy,
                                     scale=S_W)
                nc.vector.scalar_tensor_tensor(
                    hq_sb[:, :, C:C2], psum_hw[:], S_W, hq_sb[:, :, 0:C],
                    mybir.AluOpType.mult, mybir.AluOpType.subtract)
                gather_pair(hq_sb, hw2f_sb, "hw2")
                warm_pe(ps3, ph3, hq_sb[0:2 * C, 0, :],
                        adjn_res[0][0:2 * C, 0, 0:512])

            # ============ phase 4: logits sweep + transposed softmax -> E
            with tc.tile_pool(name="sm", bufs=1) as sm, \
                 tc.tile_pool(name="ps4", bufs=1, space="PSUM") as ps4:
                psum_l = ps4.tile([2 * C, ML], f32, name="psum_l",
                                  tag="ph4big")
                for dtile in range(DT):
                    for (n0, n1) in CH:
                        nc.tensor.matmul(
                            psum_l[:, n0:n1],
                            hw2f_sb[:, dtile],
                            adjn_res[dtile][:, :, n0:n1],
                            start=(dtile == 0), stop=(dtile == DT - 1),
                            perf_mode=DR)
                # start filling the adjr cache (runs during softmax + E AG)
                for dtile in range(R_RES):
                    nc.sync.dma_start(adjr_res[dtile][:], adjr_r[:, dtile])
                # PE warm pulses across the softmax's ACT/DVE chain
                warm_pe(ps4, sm, hw2f_sb[0:2 * C, 0, 0, :],
                        adjn_res[0][0:2 * C, 0, 0:512], n=8)
                # copy pair-logits to SBUF (ACT and DVE halves in parallel),
                # transpose to K-major, then softmax as fast 128-wide ops.
                lsb = sm.tile([2 * C, ML], f32)
                nc.scalar.activation(lsb[:, 0:640], psum_l[:, 0:640], AF.Copy)
                nc.vector.tensor_copy(lsb[:, 640:ML], psum_l[:, 640:ML])
                psum_t = ps4.tile([P, MT, C2], f32, name="psum_t")
                for m in range(MT):
                    nc.tensor.transpose(psum_t[:, m, :],
                                        lsb[:, m * P:(m + 1) * P],
                                        ident_sb[:])
                # ltk = strip1 + strip2 + b2 (all [128, MT, C])
                s2k = sm.tile([P, MT, C], f32)
                nc.scalar.activation(s2k[:], psum_t[:, :, C:C2], AF.Copy)
                ltk = sm.tile([P, MT, C], f32)
                nc.vector.tensor_add(ltk[:], psum_t[:, :, 0:C], s2k[:])
                ltk2 = sm.tile([P, MT, C], f32)
                nc.vector.tensor_add(ltk2[:], ltk[:], b2k_sb[:])
                expk = sm.tile([P, MT, C], f32)
                nc.scalar.activation(expk[:], ltk2[:], AF.Exp,
                                     scale=1.0 / (S_W * S_N))
                sumk = sm.tile([P, MT, 1], f32)
                nc.vector.tensor_reduce(sumk[:], expk[:],
                                        mybir.AxisListType.X,
                                        mybir.AluOpType.add)
                rck = sm.tile([P, MT, 1], f32)
                nc.vector.reciprocal(rck[:], sumk[:])
                # pb = expk * rck (broadcast over class dim); E = pb - 1/C
                pbk = sm.tile([P, MT, C], f32)
                ea, eb = bass_mod.broadcast_tensor_aps(expk[:], rck[:])
                nc.vector.tensor_mul(pbk[:], ea, eb)
                nc.scalar.activation(e_kmaj[:], pbk[:], AF.Copy,
                                     bias=-1.0 / C)

            # ============ phase 5: post-process iterations
            with tc.tile_pool(name="ph5", bufs=1) as ph5, \
                 tc.tile_pool(name="ph5d", bufs=2) as ph5d, \
                 tc.tile_pool(name="ps5z", bufs=1, space="PSUM") as ps5z, \
                 tc.tile_pool(name="ps5b", bufs=2, space="PSUM") as ps5b:
                # pre-scaled E tiles (off the iteration critical path)
                es_all = []
                for it in range(1, NPOST):
                    es = ph5.tile([P, MT, C], f32, name=f"es{it}")
                    nc.scalar.activation(es[:], e_kmaj[:], AF.Copy,
                                         scale=S_B[it])
                    es_all.append(es)
                # initial B pair = quantized E
                bq0 = ph5d.tile([P, MT, C2], f8, name="bq")
                nc.scalar.activation(bq0[:, :, 0:C], e_kmaj[:], AF.Copy,
                                     scale=S_B[0])
                nc.vector.scalar_tensor_tensor(
                    bq0[:, :, C:C2], e_kmaj[:], S_B[0], bq0[:, :, 0:C],
                    mybir.AluOpType.mult, mybir.AluOpType.subtract)
                gather_pair(bq0, btq_sb, "eb")
                warm_pe(ps5b, ph5, bq0[0:2 * C, 0, :],
                        adjn_res[0][0:2 * C, 0, 0:512])

                for it in range(NPOST):
                    # streamed adjr dts (>= R_RES): rolling 5-deep prefetch,
                    # interleaved among cached ones
                    order5 = _mix_order(R_RES, DT)
                    slot5 = {}

                    def issue5(d):
                        t = strm.tile([P, 2, ML], f8, name="adj_sl")
                        nc.sync.dma_start(t[:], adjr_r[:, d])
                        slot5[d] = t
                    pend5 = [d for d in order5 if d >= R_RES]
                    for d in pend5[:5]:
                        issue5(d)
                    n5 = min(5, len(pend5))
                    # Z = (R*S_R) @ (B pair): c-major [2C, ML] psum
                    psum_z = ps5z.tile([2 * C, ML], f32, name="psum_z")
                    for di, dtile in enumerate(order5):
                        if dtile < R_RES:
                            src = adjr_res[dtile]
                        else:
                            src = slot5.pop(dtile)
                            if n5 < len(pend5):
                                issue5(pend5[n5])
                                n5 += 1
                        for (n0, n1) in CH:
                            nc.tensor.matmul(
                                psum_z[:, n0:n1],
                                btq_sb[:, dtile],
                                src[:, :, n0:n1],
                                start=(di == 0), stop=(di == DT - 1),
                                perf_mode=DR)
                    # psum -> SBUF in 256-col chunks alternating ACT/DVE so
                    # the first H-apply matmuls start ~0.4us after the sweep
                    for ci in range(5):
                        c0, c1 = ci * 256, (ci + 1) * 256
                        if ci % 2 == 0:
                            nc.scalar.activation(zb_sb[:, c0:c1],
                                                 psum_z[:, c0:c1], AF.Copy)
                        else:
                            nc.vector.tensor_copy(zb_sb[:, c0:c1],
                                                  psum_z[:, c0:c1])
                    # B_next = E + (Z @ [H;H]) with scales folded; K-major
                    psum_bk = ps5b.tile([P, MT, C], f32, name="psum_bk")
                    for m in range(MT):
                        nc.tensor.matmul(
                            psum_bk[:, m, :],
                            zb_sb[:, m * P:(m + 1) * P],
                            hs_sb[:, it, :],
                            start=True, stop=True)
                    if it < NPOST - 1:
                        tq = ph5d.tile([P, MT, C], f32, name="tq")
                        nc.vector.tensor_add(tq[:], psum_bk[:],
                                             es_all[it][:])
                        bq = ph5d.tile([P, MT, C2], f8, name="bq")
                        nc.scalar.activation(bq[:, :, 0:C], tq[:], AF.Copy)
                        nc.vector.tensor_sub(bq[:, :, C:C2], tq[:],
                                             bq[:, :, 0:C])
                        gather_pair(bq, btq_sb, f"bt{it}")
                        warm_pe(ps5b, ph5, bq[0:2 * C, 0, :],
                                adjr_res[0][0:2 * C, 0, 0:512])
                    else:
                        bfin = ph5.tile([P, MT, C], f32)
                        nc.vector.tensor_add(bfin[:], psum_bk[:], e_kmaj[:])
                        ofin = ph5.tile([P, MT, C], f32)
                        nc.scalar.activation(ofin[:], bfin[:], AF.Copy,
                                             bias=1.0 / C)
                        nc.scalar.dma_start(
                            outKM.rearrange("(mt p) c -> p mt c", p=P),
                            ofin[:])

            adjr_cm.__exit__(None, None, None)

    nc.compile()
    return nc


def _get_compiled():
    if "nc" not in _CACHE:
        _CACHE["nc"] = _build_and_compile()
    return _CACHE["nc"]


def _prep_inputs(raw_adj, normed_adj, features, W1, b1, W2, b2, H):
    bf = ml_dtypes.bfloat16
    f8 = ml_dtypes.float8_e4m3
    f32 = np.float32

    xTfull = np.zeros((F, NK), dtype=bf)
    xTfull[:, :NREAL] = np.ascontiguousarray(features.T).astype(bf)
    w1b = np.ascontiguousarray(W1).astype(bf)
    w2b = np.ascontiguousarray(W2).astype(bf)
    b1c = np.asarray(b1, dtype=f32).reshape(HID, 1).copy()

    # hs[it] = [H; H] * (S_B[it+1] / (S_R * S_B[it])), last iter unscaled out
    Hf = np.asarray(H, dtype=np.float64)
    hs = np.zeros((NPOST, 2 * C, C), dtype=f32)
    for it in range(NPOST):
        nxt = S_B[it + 1] if it + 1 < NPOST else 1.0
        alpha = nxt / (S_R * S_B[it])
        hs[it, 0:C] = (Hf * alpha).astype(f32)
        hs[it, C:2 * C] = (Hf * alpha).astype(f32)
    hs = hs.reshape(NPOST * 2 * C, C)

    ident32 = np.eye(2 * C, dtype=f32)
    b2kfull = np.broadcast_to(
        np.asarray(b2, dtype=f32).reshape(1, 1, C), (P, MT, C))
    b2kfull = np.ascontiguousarray(b2kfull).reshape(P, MT * C)

    in_maps = []
    for r in range(RANKS):
        r0 = r * ML
        r1 = min(r0 + ML, NREAL)
        nr = r1 - r0
        an = np.zeros((NK, ML), dtype=f8)
        an[:NREAL, :nr] = (
            np.ascontiguousarray(normed_adj[r0:r1].T) * S_N).astype(f8)
        ar = np.zeros((NK, ML), dtype=f8)
        ar[:NREAL, :nr] = (
            np.ascontiguousarray(raw_adj[r0:r1].T) * S_R).astype(f8)
        in_maps.append({
            "adjn8": an, "adjr8": ar,
            "xTs": np.ascontiguousarray(xTfull[:, r0:r0 + ML]),
            "w1": w1b, "w2": w2b,
            "hs": hs, "b1": b1c, "b2k": b2kfull, "ident": ident32,
        })
    return in_maps


def run_on_device(in_maps, trace=False):
    from concourse import bass_utils
    nc = _get_compiled()
    return bass_utils.run_bass_kernel_spmd(
        nc, in_maps, core_ids=list(range(RANKS)), trace=trace)


def kernel(raw_adj, normed_adj, features, y_onehot, train_mask,
           W1, b1, W2, b2, H):
    in_maps = _prep_inputs(np.asarray(raw_adj), np.asarray(normed_adj),
                           np.asarray(features), np.asarray(W1),
                           np.asarray(b1), np.asarray(W2), np.asarray(b2),
                           np.asarray(H))
    res = run_on_device(in_maps)
    parts = []
    for r in range(RANKS):
        o = np.asarray(res.results[r]["outKM"], dtype=np.float32)  # [ML, C]
        parts.append(o)
    full = np.concatenate(parts, axis=0)[:NREAL]
    return np.ascontiguousarray(full).astype(np.float32)


# revision 90
# speedup vs baseline: 1.2041x; 1.2041x over previous
"""CPGNN (compatibility-guided GNN) kernel for 8 Trainium2 NeuronCores.

Reference computation (N=10000, F=512, HID=256, C=16, 4 post iterations):
    h      = relu(normed_adj @ (features @ W1) + b1)
    logits = normed_adj @ (h @ W2) + b2
    E_hat  = softmax(logits) - 1/C
    B_hat  = E_hat;  4x: B_hat = E_hat + raw_adj @ (B_hat @ H)
    out    = B_hat + 1/C

Strategy (v2):
  * Row-shard both adjacencies over the 8 cores (1280 rows/core, padded).
    Shards are uploaded K-major ([10240, 1280]) in fp8-e4m3 with power-of-2
    pre-scaling, and every big N x N sweep runs as fp8 DoubleRow matmuls
    (256-deep contraction per instruction, 2x PE throughput).
  * Stationary operands (XW1, hW2, B_hat) are quantized as fp8 residual
    PAIRS (q1 = fp8(x), q2 = fp8(x - q1)) placed side by side in the
    stationary, which costs nothing extra in the moving-bound sweeps and is
    more accurate than bf16.
  * raw_adj @ (B @ H) is computed as (raw_adj @ B) @ H: the narrow sweep
    produces Z = R@B c-major, then one tiny f32r matmul per m-tile applies
    [H;H] (summing the q1/q2 parts) with all de/re-quantization scales
    folded in, directly in K-major layout for the next iteration.
  * adjn is fully SBUF-resident (loaded once, reused by phases 2 and 4);
    adjr is cached during iteration 0's stream (R_RES double-tiles) and the
    rest re-streamed per iteration, when the DMA pipe is otherwise idle.
  * Phase 1 (X @ W1, bf16) is sharded: each core computes its 1280-row
    slice from a per-core xT shard and the fp8 pair is all-gathered.
  * The small [N,C]-pair tensors are all-gathered (40KB fp8/rank) between
    iterations, with PE warm-keeper chains bridging the collectives and
    p-major gather layouts for fast reloads.
"""

import os

import numpy as np
import ml_dtypes

RANKS = 8
P = 128
NREAL = 10000
NK = 10240            # padded global row count (80 k-tiles, 40 double-tiles)
ML = 1280             # local rows per core (10 m-tiles)
KT = NK // P          # 80
DT = KT // 2          # 40 double (k=256) tiles
MT = ML // P          # 10
F = 512
FT = F // P           # 4
HID = 256
C = 16
C2 = 2 * C
NPOST = 4
CH = [(0, 512), (512, 1024), (1024, 1280)]

A_RES = 40            # adjn fully resident in SBUF (loaded once, ph2 + ph4)
R_RES = 27            # adjr double-tiles cached during iter 0's stream

# power-of-2 quantization scales (validated against reference numerics)
S_N = 4096.0          # normed_adj pre-scale (host)
S_R = 8192.0          # raw_adj pre-scale (host)
S_X = 16.0            # XW1 pair scale
S_W = 64.0            # hW2 pair scale
S_B = [65536.0, 2048.0, 128.0, 8.0]   # B_hat pair scale at iteration input

# build-time switch: single-core TimelineSim build (collectives -> local DMA)
TLSIM = os.environ.get("CPGNN_TLSIM", "0") == "1"

_CACHE = {}


def _mix_order(n_cache, n_total):
    """Interleave cached (0..n_cache-1) and streamed (n_cache..) indices so
    DMA of streamed tiles overlaps PE work on cached tiles evenly."""
    cached = list(range(n_cache))
    streamed = list(range(n_cache, n_total))
    order = []
    ic = si = 0
    for i in range(n_total):
        want_stream = streamed and (si + 1) / len(streamed) <= (i + 1) / n_total
        if si < len(streamed) and (ic >= len(cached) or want_stream):
            order.append(streamed[si]); si += 1
        else:
            order.append(cached[ic]); ic += 1
    assert sorted(order) == list(range(n_total))
    return order


def _build_and_compile():
    import concourse.mybir as mybir
    import concourse.tile as tile
    import concourse.bass as bass_mod
    from concourse import bacc

    dt = mybir.dt
    f32 = dt.float32
    bf16 = dt.bfloat16
    f8 = dt.float8e4
    AF = mybir.ActivationFunctionType
    DR = mybir.MatmulPerfMode.DoubleRow

    nc = bacc.Bacc("TRN2", target_bir_lowering=False, debug=False,
                   num_devices=1 if TLSIM else RANKS)

    def all_gather(loc, full, name):
        """AllGather p-major pair shards [P, MT*C2] -> [P*RANKS, MT*C2];
        local-DMA stub under TLSIM."""
        if TLSIM:
            nc.scalar.dma_start(full[0:P, :], loc[:])
        else:
            nc.gpsimd.collective_compute(
                "AllGather", mybir.AluOpType.bypass,
                replica_groups=[list(range(RANKS))],
                ins=[loc[:].opt()], outs=[full[:].opt()])

    adjn8 = nc.dram_tensor("adjn8", [NK, ML], f8, kind="ExternalInput").ap()
    adjr8 = nc.dram_tensor("adjr8", [NK, ML], f8, kind="ExternalInput").ap()
    xTs = nc.dram_tensor("xTs", [F, ML], bf16, kind="ExternalInput").ap()
    w1 = nc.dram_tensor("w1", [F, HID], bf16, kind="ExternalInput").ap()
    w2 = nc.dram_tensor("w2", [HID, C], bf16, kind="ExternalInput").ap()
    hs = nc.dram_tensor("hs", [NPOST * 2 * C, C], f32,
                        kind="ExternalInput").ap()
    b1 = nc.dram_tensor("b1", [HID, 1], f32, kind="ExternalInput").ap()
    b2k = nc.dram_tensor("b2k", [P, MT * C], f32, kind="ExternalInput").ap()
    ident = nc.dram_tensor("ident", [2 * C, 2 * C], f32,
                           kind="ExternalInput").ap()
    outKM = nc.dram_tensor("outKM", [ML, C], f32, kind="ExternalOutput").ap()

    adjn_r = adjn8.rearrange("(t i p) m -> p t i m", i=2, p=P)
    adjr_r = adjr8.rearrange("(t i p) m -> p t i m", i=2, p=P)

    with tile.TileContext(nc) as tc:
        with tc.tile_pool(name="const", bufs=1) as const_pool, \
             tc.tile_pool(name="persist", bufs=1) as persist, \
             tc.tile_pool(name="adjn_cm", bufs=1) as adjn_cm, \
             tc.tile_pool(name="strm", bufs=5) as strm, \
             tc.tile_pool(name="dram", bufs=1, space="DRAM") as dram_pool:

            # ---- constants ----
            # w1 on the SP queue (needed immediately); the small phase-3/4/5
            # constants go on the Activation HWDGE queue to keep SP free for
            # the big streams.
            w1_sb = const_pool.tile([P, FT, HID], bf16)
            nc.scalar.dma_start(w1_sb[:],
                                w1.rearrange("(kf p) h -> p kf h", p=P))

            # ---- persistent intermediates ----
            h1t_sb = persist.tile([P, 2, ML], bf16)        # h.T  [HID, ML]
            hw2f_sb = persist.tile([P, DT, 2, C2], f8)     # gathered hW2 pair
            btq_sb = persist.tile([P, DT, 2, C2], f8)      # gathered B pair
            e_kmaj = persist.tile([P, MT, C], f32)         # E K-major local
            zb_sb = persist.tile([2 * C, ML], f32)         # sweep out copy

            # adjn residency (phase 2 + phase 4): one tile per double-tile so
            # a consumer only waits on its own DMA, not all 40
            adjn_res = [adjn_cm.tile([P, 2, ML], f8, name=f"adjn_res{d}")
                        for d in range(A_RES)]

            # p-major pair gather helpers: local [P, MT, C2] -> DRAM
            # [P, MT*C2] -> AG stacks ranks on dim0 -> [P*RANKS, MT*C2];
            # reload as [P, DT, 2, C2] with 8x320B runs per partition.
            def gather_pair(pair_sb, dst_sb, name):
                loc = dram_pool.tile([P, MT * C2], f8, name=f"{name}_loc")
                nc.scalar.dma_start(
                    loc.rearrange("p (m c) -> p m c", c=C2), pair_sb[:])
                full = dram_pool.tile([P * RANKS, MT * C2], f8,
                                      name=f"{name}_full",
                                      addr_space="Shared")
                all_gather(loc, full, name)
                # reload on the scalar HWDGE (fast descgen; the ACT queue is
                # idle at gather boundaries), split in rank halves so the
                # sweep starts once the first 4 ranks' blocks land
                dst_v = dst_sb.rearrange("p (r j) i c -> p r j i c", r=RANKS)
                full_v = full.rearrange("(r p) (j i c) -> p r j i c",
                                        p=P, i=2, c=C2)
                HR = RANKS // 2
                nc.scalar.dma_start(dst_v[:, 0:HR], full_v[:, 0:HR])
                nc.scalar.dma_start(dst_v[:, HR:RANKS], full_v[:, HR:RANKS])

            # keep the PE array warm (HAM) during a collective: a chain of
            # tiny fp8 matmuls, each gated on an ACT copy of the previous
            # (PE->ACT->PE ~0.8us/link), spreading over the gather window.
            def warm_pe(ps_pool, sb_pool, dep, mov, n=12):
                scr = ps_pool.tile([2 * C, 512], f32, name="warm_ps", bufs=1)
                tok = sb_pool.tile([2 * C, 2 * C], f8, name="warm_tok",
                                   bufs=1)
                st = dep
                for j in range(n):
                    nc.tensor.matmul(scr[:], st, mov, start=True, stop=True)
                    nc.vector.tensor_copy(tok[:], scr[:, 0:2 * C])
                    st = tok[:]

            # ============ phase 1 (sharded): local XW1 slice = Xs @ W1,
            # fp8 residual pair, all-gathered K-major across the 8 cores.
            H2 = 2 * HID
            JL = DT // RANKS                   # 5 local double-tiles
            xw_cm = tc.tile_pool(name="xw_cm", bufs=1)
            xw_pool = xw_cm.__enter__()
            xw1q_sb = [xw_pool.tile([P, JL, 2, H2], f8, name=f"xw1q{r}")
                       for r in range(RANKS)]

            with tc.tile_pool(name="ph1p", bufs=1) as ph1p, \
                 tc.tile_pool(name="ph1w", bufs=2) as ph1w, \
                 tc.tile_pool(name="ps1", bufs=4, space="PSUM") as ps1:
                xts = ph1p.tile([P, FT, ML], bf16)
                nc.sync.dma_start(xts[:],
                                  xTs.rearrange("(kf p) n -> p kf n", p=P))
                # resident adjn prefetch, head only (tail issued on the
                # gpsimd queue after the xw1q loads so they don't hog the
                # DMA pipe during the xw gather)
                for dtile in range(15):
                    nc.sync.dma_start(adjn_res[dtile][:], adjn_r[:, dtile])
                xwloc = ph1p.tile([P, JL, 2, H2], f8)
                xw_loc_dram = dram_pool.tile([P, JL * 2 * H2], f8,
                                             name="xw_loc")
                for j in range(JL):
                    for i in range(2):
                        ktl = 2 * j + i
                        psum1 = ps1.tile([P, HID], f32, name="psum1")
                        for kf in range(FT):
                            nc.tensor.matmul(
                                psum1[:],
                                xts[:, kf, ktl * P:(ktl + 1) * P],
                                w1_sb[:, kf, :],
                                start=(kf == 0), stop=(kf == FT - 1))
                        nc.scalar.activation(xwloc[:, j, i, 0:HID], psum1[:],
                                             AF.Copy, scale=S_X)
                        nc.vector.scalar_tensor_tensor(
                            xwloc[:, j, i, HID:H2], psum1[:], S_X,
                            xwloc[:, j, i, 0:HID],
                            mybir.AluOpType.mult, mybir.AluOpType.subtract)

                nc.scalar.dma_start(
                    xw_loc_dram.rearrange("p (j i q) -> p j i q", i=2, q=H2),
                    xwloc[:])
                xw_full = dram_pool.tile([P * RANKS, JL * 2 * H2], f8,
                                         name="xw_full", addr_space="Shared")
                all_gather(xw_loc_dram, xw_full, "xw")
                # per-rank reload (separate tiles) so phase 2 starts as
                # soon as rank-0's block lands; gpsimd queue keeps ACT clear
                xwf_v = xw_full.rearrange("(r p) (j i q) -> p r j i q",
                                          p=P, i=2, q=H2)
                for r in range(RANKS):
                    nc.gpsimd.dma_start(xw1q_sb[r][:], xwf_v[:, r])
                # gate the adjn tail loads behind the first xw1q blocks via
                # DVE FIFO: g1 waits the rank-2 load; the tiny tail reads
                # queue behind g1, and each tail DMA WAR-waits its read. This
                # keeps the DMA pipe clear for the xw gather chain.
                scrap = ph1w.tile([2 * C, 2 * C], f8, name="scrap", bufs=1)
                for dtile in range(15, A_RES):
                    nc.gpsimd.memset(adjn_res[dtile][0:2, 0, 0:2], 0.0)
                nc.vector.tensor_copy(scrap[:],
                                      xw1q_sb[2][0:2 * C, 0, 0, 0:2 * C])
                for dtile in range(15, A_RES):
                    nc.vector.tensor_copy(scrap[0:2, 0:2],
                                          adjn_res[dtile][0:2, 0, 0:2])
                for dtile in range(15, A_RES):
                    nc.sync.dma_start(adjn_res[dtile][:], adjn_r[:, dtile])
                warm_pe(ps1, ph1w, xwloc[0:2 * C, 0, 0, 0:2 * C],
                        adjn_res[0][0:2 * C, 0, 0:512], n=10)

            # late-phase constants (issued here so their descriptor-gen
            # doesn't clog the Activation stream during phase 1)
            w2_sb = const_pool.tile([P, 2, C], bf16)
            nc.scalar.dma_start(w2_sb[:],
                                w2.rearrange("(mh p) c -> p mh c", p=P))
            hs_sb = const_pool.tile([2 * C, NPOST, C], f32)
            nc.scalar.dma_start(hs_sb[:],
                                hs.rearrange("(n q) c -> q n c", q=2 * C))
            b1_sb = const_pool.tile([P, 2, 1], f32)
            nc.scalar.dma_start(b1_sb[:],
                                b1.rearrange("(t p) o -> p t o", p=P))
            b2k_sb = const_pool.tile([P, MT, C], f32)
            nc.scalar.dma_start(b2k_sb[:],
                                b2k.rearrange("p (m c) -> p m c", c=C))
            ident_sb = const_pool.tile([2 * C, 2 * C], f32)
            nc.scalar.dma_start(ident_sb[:], ident[:])

            # ============ phase 2: h.T = relu((XW1 pair).T @DR adjn + b1)
            with tc.tile_pool(name="ps2", bufs=1, space="PSUM") as ps2:
                psum_h0 = ps2.tile([P, ML], f32, name="psum_h0")
                psum_h1 = ps2.tile([P, ML], f32, name="psum_h1")
                psum_h = [psum_h0, psum_h1]
                for dtile in range(DT):
                    src = adjn_res[dtile]
                    for mh in range(2):
                        for qi in range(2):
                            off = qi * HID + mh * P
                            for (n0, n1) in CH:
                                nc.tensor.matmul(
                                    psum_h[mh][:, n0:n1],
                                    xw1q_sb[dtile // JL][:, dtile % JL,
                                                         :, off:off + P],
                                    src[:, :, n0:n1],
                                    start=(dtile == 0 and qi == 0),
                                    stop=(dtile == DT - 1 and qi == 1),
                                    perf_mode=DR)
                for mh in range(2):
                    nc.scalar.activation(h1t_sb[:, mh, :], psum_h[mh][:],
                                         AF.Relu, bias=b1_sb[:, mh, :],
                                         scale=1.0 / (S_X * S_N))
            xw_cm.__exit__(None, None, None)

            # adjr cache lives in the space freed by the xw1 pair
            adjr_cm = tc.tile_pool(name="adjr_cm", bufs=1)
            adjr_pool = adjr_cm.__enter__()
            adjr_res = [adjr_pool.tile([P, 2, ML], f8, name=f"adjr_res{d}")
                        for d in range(R_RES)]

            # ============ phase 3: hW2 = h @ W2 (K-major), pair, all-gather
            with tc.tile_pool(name="ph3", bufs=1) as ph3, \
                 tc.tile_pool(name="ps3", bufs=1, space="PSUM") as ps3:
                psum_hw = ps3.tile([P, MT, C], f32)
                for m in range(MT):
                    for mh in range(2):
                        nc.tensor.matmul(
                            psum_hw[:, m, :],
                            h1t_sb[:, mh, m * P:(m + 1) * P],
                            w2_sb[:, mh, :],
                            start=(mh == 0), stop=(mh == 1))
                hq_sb = ph3.tile([P, MT, C2], f8)
                nc.scalar.activation(hq_sb[:, :, 0:C], psum_hw[:], AF.Copy,
                                     scale=S_W)
                nc.vector.scalar_tensor_tensor(
                    hq_sb[:, :, C:C2], psum_hw[:], S_W, hq_sb[:, :, 0:C],
                    mybir.AluOpType.mult, mybir.AluOpType.subtract)
                gather_pair(hq_sb, hw2f_sb, "hw2")
                warm_pe(ps3, ph3, hq_sb[0:2 * C, 0, :],
                        adjn_res[0][0:2 * C, 0, 0:512])

            # ============ phase 4: logits sweep + transposed softmax -> E
            with tc.tile_pool(name="sm", bufs=1) as sm, \
                 tc.tile_pool(name="ps4", bufs=1, space="PSUM") as ps4:
                psum_l = ps4.tile([2 * C, ML], f32, name="psum_l",
                                  tag="ph4big")
                for dtile in range(DT):
                    for (n0, n1) in CH:
                        nc.tensor.matmul(
                            psum_l[:, n0:n1],
                            hw2f_sb[:, dtile],
                            adjn_res[dtile][:, :, n0:n1],
                            start=(dtile == 0), stop=(dtile == DT - 1),
                            perf_mode=DR)
                # start filling the adjr cache (runs during softmax + E AG)
                for dtile in range(R_RES):
                    nc.sync.dma_start(adjr_res[dtile][:], adjr_r[:, dtile])
                # PE warm pulses across the softmax's ACT/DVE chain
                warm_pe(ps4, sm, hw2f_sb[0:2 * C, 0, 0, :],
                        adjn_res[0][0:2 * C, 0, 0:512], n=8)
                # copy pair-logits to SBUF (ACT and DVE halves in parallel),
                # transpose to K-major, then softmax as fast 128-wide ops.
                lsb = sm.tile([2 * C, ML], f32)
                nc.scalar.activation(lsb[:, 0:640], psum_l[:, 0:640], AF.Copy)
                nc.vector.tensor_copy(lsb[:, 640:ML], psum_l[:, 640:ML])
                psum_t = ps4.tile([P, MT, C2], f32, name="psum_t")
                for m in range(MT):
                    nc.tensor.transpose(psum_t[:, m, :],
                                        lsb[:, m * P:(m + 1) * P],
                                        ident_sb[:])
                # ltk = strip1 + strip2 + b2 (all [128, MT, C])
                s2k = sm.tile([P, MT, C], f32)
                nc.scalar.activation(s2k[:], psum_t[:, :, C:C2], AF.Copy)
                ltk = sm.tile([P, MT, C], f32)
                nc.vector.tensor_add(ltk[:], psum_t[:, :, 0:C], s2k[:])
                ltk2 = sm.tile([P, MT, C], f32)
                nc.vector.tensor_add(ltk2[:], ltk[:], b2k_sb[:])
                expk = sm.tile([P, MT, C], f32)
                nc.scalar.activation(expk[:], ltk2[:], AF.Exp,
                                     scale=1.0 / (S_W * S_N))
                sumk = sm.tile([P, MT, 1], f32)
                nc.vector.tensor_reduce(sumk[:], expk[:],
                                        mybir.AxisListType.X,
                                        mybir.AluOpType.add)
                rck = sm.tile([P, MT, 1], f32)
                nc.vector.reciprocal(rck[:], sumk[:])
                # pb = expk * rck (broadcast over class dim); E = pb - 1/C
                pbk = sm.tile([P, MT, C], f32)
                ea, eb = bass_mod.broadcast_tensor_aps(expk[:], rck[:])
                nc.vector.tensor_mul(pbk[:], ea, eb)
                nc.scalar.activation(e_kmaj[:], pbk[:], AF.Copy,
                                     bias=-1.0 / C)

            # ============ phase 5: post-process iterations
            with tc.tile_pool(name="ph5", bufs=1) as ph5, \
                 tc.tile_pool(name="ph5d", bufs=2) as ph5d, \
                 tc.tile_pool(name="ps5z", bufs=1, space="PSUM") as ps5z, \
                 tc.tile_pool(name="ps5b", bufs=2, space="PSUM") as ps5b:
                # pre-scaled E tiles (off the iteration critical path)
                es_all = []
                for it in range(1, NPOST):
                    es = ph5.tile([P, MT, C], f32, name=f"es{it}")
                    nc.scalar.activation(es[:], e_kmaj[:], AF.Copy,
                                         scale=S_B[it])
                    es_all.append(es)
                # initial B pair = quantized E
                bq0 = ph5d.tile([P, MT, C2], f8, name="bq")
                nc.scalar.activation(bq0[:, :, 0:C], e_kmaj[:], AF.Copy,
                                     scale=S_B[0])
                nc.vector.scalar_tensor_tensor(
                    bq0[:, :, C:C2], e_kmaj[:], S_B[0], bq0[:, :, 0:C],
                    mybir.AluOpType.mult, mybir.AluOpType.subtract)
                gather_pair(bq0, btq_sb, "eb")
                warm_pe(ps5b, ph5, bq0[0:2 * C, 0, :],
                        adjn_res[0][0:2 * C, 0, 0:512])

                for it in range(NPOST):
                    # streamed adjr dts (>= R_RES): rolling 5-deep prefetch,
                    # interleaved among cached ones
                    order5 = _mix_order(R_RES, DT)
                    slot5 = {}

                    def issue5(d):
                        t = strm.tile([P, 2, ML], f8, name="adj_sl")
                        nc.sync.dma_start(t[:], adjr_r[:, d])
                        slot5[d] = t
                    pend5 = [d for d in order5 if d >= R_RES]
                    for d in pend5[:5]:
                        issue5(d)
                    n5 = min(5, len(pend5))
                    # Z = (R*S_R) @ (B pair): c-major [2C, ML] psum
                    psum_z = ps5z.tile([2 * C, ML], f32, name="psum_z")
                    for di, dtile in enumerate(order5):
                        if dtile < R_RES:
                            src = adjr_res[dtile]
                        else:
                            src = slot5.pop(dtile)
                            if n5 < len(pend5):
                                issue5(pend5[n5])
                                n5 += 1
                        for (n0, n1) in CH:
                            nc.tensor.matmul(
                                psum_z[:, n0:n1],
                                btq_sb[:, dtile],
                                src[:, :, n0:n1],
                                start=(di == 0), stop=(di == DT - 1),
                                perf_mode=DR)
                    # psum -> SBUF in 256-col chunks alternating ACT/DVE so
                    # the first H-apply matmuls start ~0.4us after the sweep
                    for ci in range(5):
                        c0, c1 = ci * 256, (ci + 1) * 256
                        if ci % 2 == 0:
                            nc.scalar.activation(zb_sb[:, c0:c1],
                                                 psum_z[:, c0:c1], AF.Copy)
                        else:
                            nc.vector.tensor_copy(zb_sb[:, c0:c1],
                                                  psum_z[:, c0:c1])
                    # B_next = E + (Z @ [H;H]) with scales folded; K-major
                    psum_bk = ps5b.tile([P, MT, C], f32, name="psum_bk")
                    for m in range(MT):
                        nc.tensor.matmul(
                            psum_bk[:, m, :],
                            zb_sb[:, m * P:(m + 1) * P],
                            hs_sb[:, it, :],
                            start=True, stop=True)
                    if it < NPOST - 1:
                        tq = ph5d.tile([P, MT, C], f32, name="tq")
                        nc.vector.tensor_add(tq[:], psum_bk[:],
                                             es_all[it][:])
                        bq = ph5d.tile([P, MT, C2], f8, name="bq")
                        nc.scalar.activation(bq[:, :, 0:C], tq[:], AF.Copy)
                        nc.vector.tensor_sub(bq[:, :, C:C2], tq[:],
                                             bq[:, :, 0:C])
                        gather_pair(bq, btq_sb, f"bt{it}")
                        warm_pe(ps5b, ph5, bq[0:2 * C, 0, :],
                                adjr_res[0][0:2 * C, 0, 0:512])
                    else:
                        ofin = ph5.tile([P, MT, C], f32)
                        nc.vector.scalar_tensor_tensor(
                            ofin[:], psum_bk[:], 1.0 / C, e_kmaj[:],
                            mybir.AluOpType.add, mybir.AluOpType.add)
                        nc.scalar.dma_start(
                            outKM.rearrange("(mt p) c -> p mt c", p=P),
                            ofin[:])

            adjr_cm.__exit__(None, None, None)

    nc.compile()
    return nc


def _get_compiled():
    if "nc" not in _CACHE:
        _CACHE["nc"] = _build_and_compile()
    return _CACHE["nc"]


def _prep_inputs(raw_adj, normed_adj, features, W1, b1, W2, b2, H):
    bf = ml_dtypes.bfloat16
    f8 = ml_dtypes.float8_e4m3
    f32 = np.float32

    xTfull = np.zeros((F, NK), dtype=bf)
    xTfull[:, :NREAL] = np.ascontiguousarray(features.T).astype(bf)
    w1b = np.ascontiguousarray(W1).astype(bf)
    w2b = np.ascontiguousarray(W2).astype(bf)
    b1c = np.asarray(b1, dtype=f32).reshape(HID, 1).copy()

    # hs[it] = [H; H] * (S_B[it+1] / (S_R * S_B[it])), last iter unscaled out
    Hf = np.asarray(H, dtype=np.float64)
    hs = np.zeros((NPOST, 2 * C, C), dtype=f32)
    for it in range(NPOST):
        nxt = S_B[it + 1] if it + 1 < NPOST else 1.0
        alpha = nxt / (S_R * S_B[it])
        hs[it, 0:C] = (Hf * alpha).astype(f32)
        hs[it, C:2 * C] = (Hf * alpha).astype(f32)
    hs = hs.reshape(NPOST * 2 * C, C)

    ident32 = np.eye(2 * C, dtype=f32)
    b2kfull = np.broadcast_to(
        np.asarray(b2, dtype=f32).reshape(1, 1, C), (P, MT, C))
    b2kfull = np.ascontiguousarray(b2kfull).reshape(P, MT * C)

    in_maps = []
    for r in range(RANKS):
        r0 = r * ML
        r1 = min(r0 + ML, NREAL)
        nr = r1 - r0
        an = np.zeros((NK, ML), dtype=f8)
        an[:NREAL, :nr] = (
            np.ascontiguousarray(normed_adj[r0:r1].T) * S_N).astype(f8)
        ar = np.zeros((NK, ML), dtype=f8)
        ar[:NREAL, :nr] = (
            np.ascontiguousarray(raw_adj[r0:r1].T) * S_R).astype(f8)
        in_maps.append({
            "adjn8": an, "adjr8": ar,
            "xTs": np.ascontiguousarray(xTfull[:, r0:r0 + ML]),
            "w1": w1b, "w2": w2b,
            "hs": hs, "b1": b1c, "b2k": b2kfull, "ident": ident32,
        })
    return in_maps


def run_on_device(in_maps, trace=False):
    from concourse import bass_utils
    nc = _get_compiled()
    return bass_utils.run_bass_kernel_spmd(
        nc, in_maps, core_ids=list(range(RANKS)), trace=trace)


def kernel(raw_adj, normed_adj, features, y_onehot, train_mask,
           W1, b1, W2, b2, H):
    in_maps = _prep_inputs(np.asarray(raw_adj), np.asarray(normed_adj),
                           np.asarray(features), np.asarray(W1),
                           np.asarray(b1), np.asarray(W2), np.asarray(b2),
                           np.asarray(H))
    res = run_on_device(in_maps)
    parts = []
    for r in range(RANKS):
        o = np.asarray(res.results[r]["outKM"], dtype=np.float32)  # [ML, C]
        parts.append(o)
    full = np.concatenate(parts, axis=0)[:NREAL]
    return np.ascontiguousarray(full).astype(np.float32)


# revision 91
# speedup vs baseline: 1.2263x; 1.0184x over previous
"""CPGNN (compatibility-guided GNN) kernel for 8 Trainium2 NeuronCores.

Reference computation (N=10000, F=512, HID=256, C=16, 4 post iterations):
    h      = relu(normed_adj @ (features @ W1) + b1)
    logits = normed_adj @ (h @ W2) + b2
    E_hat  = softmax(logits) - 1/C
    B_hat  = E_hat;  4x: B_hat = E_hat + raw_adj @ (B_hat @ H)
    out    = B_hat + 1/C

Strategy (v2):
  * Row-shard both adjacencies over the 8 cores (1280 rows/core, padded).
    Shards are uploaded K-major ([10240, 1280]) in fp8-e4m3 with power-of-2
    pre-scaling, and every big N x N sweep runs as fp8 DoubleRow matmuls
    (256-deep contraction per instruction, 2x PE throughput).
  * Stationary operands (XW1, hW2, B_hat) are quantized as fp8 residual
    PAIRS (q1 = fp8(x), q2 = fp8(x - q1)) placed side by side in the
    stationary, which costs nothing extra in the moving-bound sweeps and is
    more accurate than bf16.
  * raw_adj @ (B @ H) is computed as (raw_adj @ B) @ H: the narrow sweep
    produces Z = R@B c-major, then one tiny f32r matmul per m-tile applies
    [H;H] (summing the q1/q2 parts) with all de/re-quantization scales
    folded in, directly in K-major layout for the next iteration.
  * adjn is fully SBUF-resident (loaded once, reused by phases 2 and 4);
    adjr is cached during iteration 0's stream (R_RES double-tiles) and the
    rest re-streamed per iteration, when the DMA pipe is otherwise idle.
  * Phase 1 (X @ W1, bf16) is sharded: each core computes its 1280-row
    slice from a per-core xT shard and the fp8 pair is all-gathered.
  * The small [N,C]-pair tensors are all-gathered (40KB fp8/rank) between
    iterations, with PE warm-keeper chains bridging the collectives and
    p-major gather layouts for fast reloads.
"""

import os

import numpy as np
import ml_dtypes

RANKS = 8
P = 128
NREAL = 10000
NK = 10240            # padded global row count (80 k-tiles, 40 double-tiles)
ML = 1280             # local rows per core (10 m-tiles)
KT = NK // P          # 80
DT = KT // 2          # 40 double (k=256) tiles
MT = ML // P          # 10
F = 512
FT = F // P           # 4
HID = 256
C = 16
C2 = 2 * C
NPOST = 4
CH = [(0, 512), (512, 1024), (1024, 1280)]

A_RES = 40            # adjn fully resident in SBUF (loaded once, ph2 + ph4)
R_RES = 27            # adjr double-tiles cached during iter 0's stream

# power-of-2 quantization scales (validated against reference numerics)
S_N = 4096.0          # normed_adj pre-scale (host)
S_R = 8192.0          # raw_adj pre-scale (host)
S_X = 16.0            # XW1 pair scale
S_W = 64.0            # hW2 pair scale
S_B = [65536.0, 2048.0, 128.0, 8.0]   # B_hat pair scale at iteration input

# build-time switch: single-core TimelineSim build (collectives -> local DMA)
TLSIM = os.environ.get("CPGNN_TLSIM", "0") == "1"

_CACHE = {}


def _mix_order(n_cache, n_total):
    """Interleave cached (0..n_cache-1) and streamed (n_cache..) indices so
    DMA of streamed tiles overlaps PE work on cached tiles evenly."""
    cached = list(range(n_cache))
    streamed = list(range(n_cache, n_total))
    order = []
    ic = si = 0
    for i in range(n_total):
        want_stream = streamed and (si + 1) / len(streamed) <= (i + 1) / n_total
        if si < len(streamed) and (ic >= len(cached) or want_stream):
            order.append(streamed[si]); si += 1
        else:
            order.append(cached[ic]); ic += 1
    assert sorted(order) == list(range(n_total))
    return order


def _build_and_compile():
    import concourse.mybir as mybir
    import concourse.tile as tile
    import concourse.bass as bass_mod
    from concourse import bacc

    dt = mybir.dt
    f32 = dt.float32
    bf16 = dt.bfloat16
    f8 = dt.float8e4
    AF = mybir.ActivationFunctionType
    DR = mybir.MatmulPerfMode.DoubleRow

    nc = bacc.Bacc("TRN2", target_bir_lowering=False, debug=False,
                   num_devices=1 if TLSIM else RANKS)

    def all_gather(loc, full, name):
        """AllGather p-major pair shards [P, MT*C2] -> [P*RANKS, MT*C2];
        local-DMA stub under TLSIM."""
        if TLSIM:
            nc.scalar.dma_start(full[0:P, :], loc[:])
        else:
            nc.gpsimd.collective_compute(
                "AllGather", mybir.AluOpType.bypass,
                replica_groups=[list(range(RANKS))],
                ins=[loc[:].opt()], outs=[full[:].opt()])

    adjn8 = nc.dram_tensor("adjn8", [NK, ML], f8, kind="ExternalInput").ap()
    adjr8 = nc.dram_tensor("adjr8", [NK, ML], f8, kind="ExternalInput").ap()
    xTs = nc.dram_tensor("xTs", [F, ML], bf16, kind="ExternalInput").ap()
    w1 = nc.dram_tensor("w1", [F, HID], bf16, kind="ExternalInput").ap()
    w2 = nc.dram_tensor("w2", [HID, C], bf16, kind="ExternalInput").ap()
    hs = nc.dram_tensor("hs", [NPOST * 2 * C, C], f32,
                        kind="ExternalInput").ap()
    b1 = nc.dram_tensor("b1", [HID, 1], f32, kind="ExternalInput").ap()
    b2k = nc.dram_tensor("b2k", [P, MT * C], f32, kind="ExternalInput").ap()
    ident = nc.dram_tensor("ident", [2 * C, 2 * C], f32,
                           kind="ExternalInput").ap()
    outKM = nc.dram_tensor("outKM", [ML, C], f32, kind="ExternalOutput").ap()

    adjn_r = adjn8.rearrange("(t i p) m -> p t i m", i=2, p=P)
    adjr_r = adjr8.rearrange("(t i p) m -> p t i m", i=2, p=P)

    with tile.TileContext(nc) as tc:
        with tc.tile_pool(name="const", bufs=1) as const_pool, \
             tc.tile_pool(name="persist", bufs=1) as persist, \
             tc.tile_pool(name="adjn_cm", bufs=1) as adjn_cm, \
             tc.tile_pool(name="strm", bufs=5) as strm, \
             tc.tile_pool(name="dram", bufs=1, space="DRAM") as dram_pool:

            # ---- constants ----
            # w1 on the SP queue (needed immediately); the small phase-3/4/5
            # constants go on the Activation HWDGE queue to keep SP free for
            # the big streams.
            w1_sb = const_pool.tile([P, FT, HID], bf16)
            nc.scalar.dma_start(w1_sb[:],
                                w1.rearrange("(kf p) h -> p kf h", p=P))

            # ---- persistent intermediates ----
            h1t_sb = persist.tile([P, 2, ML], bf16)        # h.T  [HID, ML]
            hw2f_sb = persist.tile([P, DT, 2, C2], f8)     # gathered hW2 pair
            btq_sb = persist.tile([P, DT, 2, C2], f8)      # gathered B pair
            e_kmaj = persist.tile([P, MT, C], f32)         # E K-major local
            zb_sb = persist.tile([2 * C, ML], f32)         # sweep out copy

            # adjn residency (phase 2 + phase 4): one tile per double-tile so
            # a consumer only waits on its own DMA, not all 40
            adjn_res = [adjn_cm.tile([P, 2, ML], f8, name=f"adjn_res{d}")
                        for d in range(A_RES)]

            # p-major pair gather helpers: local [P, MT, C2] -> DRAM
            # [P, MT*C2] -> AG stacks ranks on dim0 -> [P*RANKS, MT*C2];
            # reload as [P, DT, 2, C2] with 8x320B runs per partition.
            def gather_pair(pair_sb, dst_sb, name):
                loc = dram_pool.tile([P, MT * C2], f8, name=f"{name}_loc")
                nc.scalar.dma_start(
                    loc.rearrange("p (m c) -> p m c", c=C2), pair_sb[:])
                full = dram_pool.tile([P * RANKS, MT * C2], f8,
                                      name=f"{name}_full",
                                      addr_space="Shared")
                all_gather(loc, full, name)
                # reload on the scalar HWDGE (fast descgen; the ACT queue is
                # idle at gather boundaries), split in rank halves so the
                # sweep starts once the first 4 ranks' blocks land
                dst_v = dst_sb.rearrange("p (r j) i c -> p r j i c", r=RANKS)
                full_v = full.rearrange("(r p) (j i c) -> p r j i c",
                                        p=P, i=2, c=C2)
                HR = RANKS // 2
                nc.scalar.dma_start(dst_v[:, 0:HR], full_v[:, 0:HR])
                nc.scalar.dma_start(dst_v[:, HR:RANKS], full_v[:, HR:RANKS])

            # keep the PE array warm (HAM) during a collective: a chain of
            # tiny fp8 matmuls, each gated on an ACT copy of the previous
            # (PE->ACT->PE ~0.8us/link), spreading over the gather window.
            def warm_pe(ps_pool, sb_pool, dep, mov, n=12):
                scr = ps_pool.tile([2 * C, 512], f32, name="warm_ps", bufs=1)
                tok = sb_pool.tile([2 * C, 2 * C], f8, name="warm_tok",
                                   bufs=1)
                st = dep
                for j in range(n):
                    nc.tensor.matmul(scr[:], st, mov, start=True, stop=True)
                    nc.vector.tensor_copy(tok[:], scr[:, 0:2 * C])
                    st = tok[:]

            # ============ phase 1 (sharded): local XW1 slice = Xs @ W1,
            # fp8 residual pair, all-gathered K-major across the 8 cores.
            H2 = 2 * HID
            JL = DT // RANKS                   # 5 local double-tiles
            xw_cm = tc.tile_pool(name="xw_cm", bufs=1)
            xw_pool = xw_cm.__enter__()
            xw1q_sb = [xw_pool.tile([P, JL, 2, H2], f8, name=f"xw1q{r}")
                       for r in range(RANKS)]

            with tc.tile_pool(name="ph1p", bufs=1) as ph1p, \
                 tc.tile_pool(name="ph1w", bufs=2) as ph1w, \
                 tc.tile_pool(name="ps1", bufs=4, space="PSUM") as ps1:
                xts = ph1p.tile([P, FT, ML], bf16)
                nc.sync.dma_start(xts[:],
                                  xTs.rearrange("(kf p) n -> p kf n", p=P))
                # resident adjn prefetch, head only (tail issued on the
                # gpsimd queue after the xw1q loads so they don't hog the
                # DMA pipe during the xw gather)
                for dtile in range(15):
                    nc.sync.dma_start(adjn_res[dtile][:], adjn_r[:, dtile])
                xwloc = ph1p.tile([P, JL, 2, H2], f8)
                xw_loc_dram = dram_pool.tile([P, JL * 2 * H2], f8,
                                             name="xw_loc")
                for j in range(JL):
                    for i in range(2):
                        ktl = 2 * j + i
                        psum1 = ps1.tile([P, HID], f32, name="psum1")
                        for kf in range(FT):
                            nc.tensor.matmul(
                                psum1[:],
                                xts[:, kf, ktl * P:(ktl + 1) * P],
                                w1_sb[:, kf, :],
                                start=(kf == 0), stop=(kf == FT - 1))
                        nc.scalar.activation(xwloc[:, j, i, 0:HID], psum1[:],
                                             AF.Copy, scale=S_X)
                        nc.vector.scalar_tensor_tensor(
                            xwloc[:, j, i, HID:H2], psum1[:], S_X,
                            xwloc[:, j, i, 0:HID],
                            mybir.AluOpType.mult, mybir.AluOpType.subtract)

                nc.scalar.dma_start(
                    xw_loc_dram.rearrange("p (j i q) -> p j i q", i=2, q=H2),
                    xwloc[:])
                xw_full = dram_pool.tile([P * RANKS, JL * 2 * H2], f8,
                                         name="xw_full", addr_space="Shared")
                all_gather(xw_loc_dram, xw_full, "xw")
                # per-rank reload (separate tiles) so phase 2 starts as
                # soon as rank-0's block lands; scalar HWDGE has the fastest
                # descgen and its queue is clear here (consts moved later,
                # warm chain runs on DVE)
                xwf_v = xw_full.rearrange("(r p) (j i q) -> p r j i q",
                                          p=P, i=2, q=H2)
                for r in range(RANKS):
                    nc.scalar.dma_start(xw1q_sb[r][:], xwf_v[:, r])
                # gate the adjn tail loads behind the first xw1q blocks via
                # DVE FIFO: g1 waits the rank-2 load; the tiny tail reads
                # queue behind g1, and each tail DMA WAR-waits its read. This
                # keeps the DMA pipe clear for the xw gather chain.
                scrap = ph1w.tile([2 * C, 2 * C], f8, name="scrap", bufs=1)
                for dtile in range(15, A_RES):
                    nc.gpsimd.memset(adjn_res[dtile][0:2, 0, 0:2], 0.0)
                nc.vector.tensor_copy(scrap[:],
                                      xw1q_sb[2][0:2 * C, 0, 0, 0:2 * C])
                for dtile in range(15, A_RES):
                    nc.vector.tensor_copy(scrap[0:2, 0:2],
                                          adjn_res[dtile][0:2, 0, 0:2])
                for dtile in range(15, A_RES):
                    nc.sync.dma_start(adjn_res[dtile][:], adjn_r[:, dtile])
                warm_pe(ps1, ph1w, xwloc[0:2 * C, 0, 0, 0:2 * C],
                        adjn_res[0][0:2 * C, 0, 0:512], n=10)

            # late-phase constants (issued here so their descriptor-gen
            # doesn't clog the Activation stream during phase 1)
            w2_sb = const_pool.tile([P, 2, C], bf16)
            nc.scalar.dma_start(w2_sb[:],
                                w2.rearrange("(mh p) c -> p mh c", p=P))
            hs_sb = const_pool.tile([2 * C, NPOST, C], f32)
            nc.scalar.dma_start(hs_sb[:],
                                hs.rearrange("(n q) c -> q n c", q=2 * C))
            b1_sb = const_pool.tile([P, 2, 1], f32)
            nc.scalar.dma_start(b1_sb[:],
                                b1.rearrange("(t p) o -> p t o", p=P))
            b2k_sb = const_pool.tile([P, MT, C], f32)
            nc.scalar.dma_start(b2k_sb[:],
                                b2k.rearrange("p (m c) -> p m c", c=C))
            ident_sb = const_pool.tile([2 * C, 2 * C], f32)
            nc.scalar.dma_start(ident_sb[:], ident[:])

            # ============ phase 2: h.T = relu((XW1 pair).T @DR adjn + b1)
            with tc.tile_pool(name="ps2", bufs=1, space="PSUM") as ps2:
                psum_h0 = ps2.tile([P, ML], f32, name="psum_h0")
                psum_h1 = ps2.tile([P, ML], f32, name="psum_h1")
                psum_h = [psum_h0, psum_h1]
                for dtile in range(DT):
                    src = adjn_res[dtile]
                    for mh in range(2):
                        for qi in range(2):
                            off = qi * HID + mh * P
                            for (n0, n1) in CH:
                                nc.tensor.matmul(
                                    psum_h[mh][:, n0:n1],
                                    xw1q_sb[dtile // JL][:, dtile % JL,
                                                         :, off:off + P],
                                    src[:, :, n0:n1],
                                    start=(dtile == 0 and qi == 0),
                                    stop=(dtile == DT - 1 and qi == 1),
                                    perf_mode=DR)
                for mh in range(2):
                    nc.scalar.activation(h1t_sb[:, mh, :], psum_h[mh][:],
                                         AF.Relu, bias=b1_sb[:, mh, :],
                                         scale=1.0 / (S_X * S_N))
            xw_cm.__exit__(None, None, None)

            # adjr cache lives in the space freed by the xw1 pair
            adjr_cm = tc.tile_pool(name="adjr_cm", bufs=1)
            adjr_pool = adjr_cm.__enter__()
            adjr_res = [adjr_pool.tile([P, 2, ML], f8, name=f"adjr_res{d}")
                        for d in range(R_RES)]

            # ============ phase 3: hW2 = h @ W2 (K-major), pair, all-gather
            with tc.tile_pool(name="ph3", bufs=1) as ph3, \
                 tc.tile_pool(name="ps3", bufs=1, space="PSUM") as ps3:
                psum_hw = ps3.tile([P, MT, C], f32)
                for m in range(MT):
                    for mh in range(2):
                        nc.tensor.matmul(
                            psum_hw[:, m, :],
                            h1t_sb[:, mh, m * P:(m + 1) * P],
                            w2_sb[:, mh, :],
                            start=(mh == 0), stop=(mh == 1))
                hq_sb = ph3.tile([P, MT, C2], f8)
                nc.scalar.activation(hq_sb[:, :, 0:C], psum_hw[:], AF.Copy,
                                     scale=S_W)
                nc.vector.scalar_tensor_tensor(
                    hq_sb[:, :, C:C2], psum_hw[:], S_W, hq_sb[:, :, 0:C],
                    mybir.AluOpType.mult, mybir.AluOpType.subtract)
                gather_pair(hq_sb, hw2f_sb, "hw2")
                warm_pe(ps3, ph3, hq_sb[0:2 * C, 0, :],
                        adjn_res[0][0:2 * C, 0, 0:512])

            # ============ phase 4: logits sweep + transposed softmax -> E
            with tc.tile_pool(name="sm", bufs=1) as sm, \
                 tc.tile_pool(name="ps4", bufs=1, space="PSUM") as ps4:
                psum_l = ps4.tile([2 * C, ML], f32, name="psum_l",
                                  tag="ph4big")
                for dtile in range(DT):
                    for (n0, n1) in CH:
                        nc.tensor.matmul(
                            psum_l[:, n0:n1],
                            hw2f_sb[:, dtile],
                            adjn_res[dtile][:, :, n0:n1],
                            start=(dtile == 0), stop=(dtile == DT - 1),
                            perf_mode=DR)
                # start filling the adjr cache (runs during softmax + E AG)
                for dtile in range(R_RES):
                    nc.sync.dma_start(adjr_res[dtile][:], adjr_r[:, dtile])
                # PE warm pulses across the softmax's ACT/DVE chain
                warm_pe(ps4, sm, hw2f_sb[0:2 * C, 0, 0, :],
                        adjn_res[0][0:2 * C, 0, 0:512], n=8)
                # copy pair-logits to SBUF (ACT and DVE halves in parallel),
                # transpose to K-major, then softmax as fast 128-wide ops.
                lsb = sm.tile([2 * C, ML], f32)
                nc.scalar.activation(lsb[:, 0:640], psum_l[:, 0:640], AF.Copy)
                nc.vector.tensor_copy(lsb[:, 640:ML], psum_l[:, 640:ML])
                psum_t = ps4.tile([P, MT, C2], f32, name="psum_t")
                for m in range(MT):
                    nc.tensor.transpose(psum_t[:, m, :],
                                        lsb[:, m * P:(m + 1) * P],
                                        ident_sb[:])
                # ltk = strip1 + strip2 + b2 (all [128, MT, C])
                s2k = sm.tile([P, MT, C], f32)
                nc.scalar.activation(s2k[:], psum_t[:, :, C:C2], AF.Copy)
                ltk = sm.tile([P, MT, C], f32)
                nc.vector.tensor_add(ltk[:], psum_t[:, :, 0:C], s2k[:])
                ltk2 = sm.tile([P, MT, C], f32)
                nc.vector.tensor_add(ltk2[:], ltk[:], b2k_sb[:])
                expk = sm.tile([P, MT, C], f32)
                nc.scalar.activation(expk[:], ltk2[:], AF.Exp,
                                     scale=1.0 / (S_W * S_N))
                sumk = sm.tile([P, MT, 1], f32)
                nc.vector.tensor_reduce(sumk[:], expk[:],
                                        mybir.AxisListType.X,
                                        mybir.AluOpType.add)
                rck = sm.tile([P, MT, 1], f32)
                nc.vector.reciprocal(rck[:], sumk[:])
                # pb = expk * rck (broadcast over class dim); E = pb - 1/C
                pbk = sm.tile([P, MT, C], f32)
                ea, eb = bass_mod.broadcast_tensor_aps(expk[:], rck[:])
                nc.vector.tensor_mul(pbk[:], ea, eb)
                nc.scalar.activation(e_kmaj[:], pbk[:], AF.Copy,
                                     bias=-1.0 / C)

            # ============ phase 5: post-process iterations
            with tc.tile_pool(name="ph5", bufs=1) as ph5, \
                 tc.tile_pool(name="ph5d", bufs=2) as ph5d, \
                 tc.tile_pool(name="ps5z", bufs=1, space="PSUM") as ps5z, \
                 tc.tile_pool(name="ps5b", bufs=2, space="PSUM") as ps5b:
                # pre-scaled E tiles (off the iteration critical path)
                es_all = []
                for it in range(1, NPOST):
                    es = ph5.tile([P, MT, C], f32, name=f"es{it}")
                    nc.scalar.activation(es[:], e_kmaj[:], AF.Copy,
                                         scale=S_B[it])
                    es_all.append(es)
                # initial B pair = quantized E
                bq0 = ph5d.tile([P, MT, C2], f8, name="bq")
                nc.scalar.activation(bq0[:, :, 0:C], e_kmaj[:], AF.Copy,
                                     scale=S_B[0])
                nc.vector.scalar_tensor_tensor(
                    bq0[:, :, C:C2], e_kmaj[:], S_B[0], bq0[:, :, 0:C],
                    mybir.AluOpType.mult, mybir.AluOpType.subtract)
                gather_pair(bq0, btq_sb, "eb")
                warm_pe(ps5b, ph5, bq0[0:2 * C, 0, :],
                        adjn_res[0][0:2 * C, 0, 0:512])

                for it in range(NPOST):
                    # streamed adjr dts (>= R_RES): rolling 5-deep prefetch,
                    # interleaved among cached ones
                    order5 = _mix_order(R_RES, DT)
                    slot5 = {}

                    def issue5(d):
                        t = strm.tile([P, 2, ML], f8, name="adj_sl")
                        nc.sync.dma_start(t[:], adjr_r[:, d])
                        slot5[d] = t
                    pend5 = [d for d in order5 if d >= R_RES]
                    for d in pend5[:5]:
                        issue5(d)
                    n5 = min(5, len(pend5))
                    # Z = (R*S_R) @ (B pair): c-major [2C, ML] psum
                    psum_z = ps5z.tile([2 * C, ML], f32, name="psum_z")
                    for di, dtile in enumerate(order5):
                        if dtile < R_RES:
                            src = adjr_res[dtile]
                        else:
                            src = slot5.pop(dtile)
                            if n5 < len(pend5):
                                issue5(pend5[n5])
                                n5 += 1
                        for (n0, n1) in CH:
                            nc.tensor.matmul(
                                psum_z[:, n0:n1],
                                btq_sb[:, dtile],
                                src[:, :, n0:n1],
                                start=(di == 0), stop=(di == DT - 1),
                                perf_mode=DR)
                    # psum -> SBUF in 256-col chunks alternating ACT/DVE so
                    # the first H-apply matmuls start ~0.4us after the sweep
                    for ci in range(5):
                        c0, c1 = ci * 256, (ci + 1) * 256
                        if ci % 2 == 0:
                            nc.scalar.activation(zb_sb[:, c0:c1],
                                                 psum_z[:, c0:c1], AF.Copy)
                        else:
                            nc.vector.tensor_copy(zb_sb[:, c0:c1],
                                                  psum_z[:, c0:c1])
                    # B_next = E + (Z @ [H;H]) with scales folded; K-major
                    psum_bk = ps5b.tile([P, MT, C], f32, name="psum_bk")
                    for m in range(MT):
                        nc.tensor.matmul(
                            psum_bk[:, m, :],
                            zb_sb[:, m * P:(m + 1) * P],
                            hs_sb[:, it, :],
                            start=True, stop=True)
                    if it < NPOST - 1:
                        tq = ph5d.tile([P, MT, C], f32, name="tq")
                        nc.vector.tensor_add(tq[:], psum_bk[:],
                                             es_all[it][:])
                        bq = ph5d.tile([P, MT, C2], f8, name="bq")
                        nc.scalar.activation(bq[:, :, 0:C], tq[:], AF.Copy)
                        nc.vector.tensor_sub(bq[:, :, C:C2], tq[:],
                                             bq[:, :, 0:C])
                        gather_pair(bq, btq_sb, f"bt{it}")
                        warm_pe(ps5b, ph5, bq[0:2 * C, 0, :],
                                adjr_res[0][0:2 * C, 0, 0:512])
                    else:
                        ofin = ph5.tile([P, MT, C], f32)
                        nc.vector.scalar_tensor_tensor(
                            ofin[:], psum_bk[:], 1.0 / C, e_kmaj[:],
                            mybir.AluOpType.add, mybir.AluOpType.add)
                        nc.scalar.dma_start(
                            outKM.rearrange("(mt p) c -> p mt c", p=P),
                            ofin[:])

            adjr_cm.__exit__(None, None, None)

    nc.compile()
    return nc


def _get_compiled():
    if "nc" not in _CACHE:
        _CACHE["nc"] = _build_and_compile()
    return _CACHE["nc"]


def _prep_inputs(raw_adj, normed_adj, features, W1, b1, W2, b2, H):
    bf = ml_dtypes.bfloat16
    f8 = ml_dtypes.float8_e4m3
    f32 = np.float32

    xTfull = np.zeros((F, NK), dtype=bf)
    xTfull[:, :NREAL] = np.ascontiguousarray(features.T).astype(bf)
    w1b = np.ascontiguousarray(W1).astype(bf)
    w2b = np.ascontiguousarray(W2).astype(bf)
    b1c = np.asarray(b1, dtype=f32).reshape(HID, 1).copy()

    # hs[it] = [H; H] * (S_B[it+1] / (S_R * S_B[it])), last iter unscaled out
    Hf = np.asarray(H, dtype=np.float64)
    hs = np.zeros((NPOST, 2 * C, C), dtype=f32)
    for it in range(NPOST):
        nxt = S_B[it + 1] if it + 1 < NPOST else 1.0
        alpha = nxt / (S_R * S_B[it])
        hs[it, 0:C] = (Hf * alpha).astype(f32)
        hs[it, C:2 * C] = (Hf * alpha).astype(f32)
    hs = hs.reshape(NPOST * 2 * C, C)

    ident32 = np.eye(2 * C, dtype=f32)
    b2kfull = np.broadcast_to(
        np.asarray(b2, dtype=f32).reshape(1, 1, C), (P, MT, C))
    b2kfull = np.ascontiguousarray(b2kfull).reshape(P, MT * C)

    in_maps = []
    for r in range(RANKS):
        r0 = r * ML
        r1 = min(r0 + ML, NREAL)
        nr = r1 - r0
        an = np.zeros((NK, ML), dtype=f8)
        an[:NREAL, :nr] = (
            np.ascontiguousarray(normed_adj[r0:r1].T) * S_N).astype(f8)
        ar = np.zeros((NK, ML), dtype=f8)
        ar[:NREAL, :nr] = (
            np.ascontiguousarray(raw_adj[r0:r1].T) * S_R).astype(f8)
        in_maps.append({
            "adjn8": an, "adjr8": ar,
            "xTs": np.ascontiguousarray(xTfull[:, r0:r0 + ML]),
            "w1": w1b, "w2": w2b,
            "hs": hs, "b1": b1c, "b2k": b2kfull, "ident": ident32,
        })
    return in_maps


def run_on_device(in_maps, trace=False):
    from concourse import bass_utils
    nc = _get_compiled()
    return bass_utils.run_bass_kernel_spmd(
        nc, in_maps, core_ids=list(range(RANKS)), trace=trace)


def kernel(raw_adj, normed_adj, features, y_onehot, train_mask,
           W1, b1, W2, b2, H):
    in_maps = _prep_inputs(np.asarray(raw_adj), np.asarray(normed_adj),
                           np.asarray(features), np.asarray(W1),
                           np.asarray(b1), np.asarray(W2), np.asarray(b2),
                           np.asarray(H))
    res = run_on_device(in_maps)
    parts = []
    for r in range(RANKS):
        o = np.asarray(res.results[r]["outKM"], dtype=np.float32)  # [ML, C]
        parts.append(o)
    full = np.concatenate(parts, axis=0)[:NREAL]
    return np.ascontiguousarray(full).astype(np.float32)
